# revision 20
# baseline (speedup 1.0000x reference)
"""Trainium2 Bass kernel for nn_AttentionWithCache (decode attention with KV cache).

Full-input contract: kernel(**inputs) takes the unsharded numpy inputs and
returns the full [1, 128, 4096] output. Internally shards tensor-parallel
over heads across 8 NeuronCores (4 heads each), runs a Bass/Tile kernel via
run_bass_kernel_spmd, and reduces the wo partial sums on gather.

Key algebraic simplification: the reference applies RoPE at a single scalar
position `pos` to BOTH q and the whole live k-cache. A per-(i, i+64) plane
rotation by the same angle on both operands of a dot product leaves the dot
product unchanged, and v is never rotated — so attention scores (and hence
the output) are mathematically identical without RoPE. The kernel skips it.

Softmax skips max-subtraction: scores are ~N(0, 1.65^2) (|s| < ~10), so
exp() cannot overflow in fp32 and softmax is shift-invariant. The softmax
denominator comes for free from a ones-column appended to v (the attn@v
matmul's extra output column is the row sum of the probabilities).
"""

import sys

if "/opt/trn_rl_repo" not in sys.path:
    sys.path.insert(0, "/opt/trn_rl_repo")

import ml_dtypes
import numpy as np

import concourse.bass as bass
import concourse.mybir as mybir
import concourse.tile as tile
from concourse import bacc
from concourse.bass import ts
from concourse.bass_utils import run_bass_kernel_spmd
from concourse.masks import make_identity

# Problem shapes (hardcoded per contract).
B, T, D = 1, 128, 4096
H, HD = 32, 128
CACHE_POS = 4096
S = CACHE_POS + T            # 4224 live cache rows
N_CORES = 8
NH = H // N_CORES            # 4 heads per core
O = NH * HD                  # 512 projection out-dims per core
NC_I = D // 128              # 32 contraction chunks for projections
NC_S = CACHE_POS // 128      # 32 old-cache s-chunks (the 33rd chunk is new k/v)
SCALE = 1.0 / float(np.sqrt(HD))
# Constant subtracted inside exp() (cancels exactly in the softmax ratio).
# Raw scores reach ~±18.5; fp16 exp overflows at 11.09, so shift down.
EXP_BIAS = -13.0

F32 = mybir.dt.float32
F32R = mybir.dt.float32r
BF16 = mybir.dt.bfloat16
F16 = mybir.dt.float16

# Precision config.
#   W_DT:     dtype of x + projection/wo weights as matmul operands
#             ('f32r' = tf32-like, 4B DMA; 'bf16' = 2B DMA; 'f32' exact 2-pass)
#   CACHE_DT: dtype of k/v caches + q/probs as attention matmul operands
W_DT = "f32r"
CACHE_DT = "f32"

TRACE = False       # set by test.py for profiling runs
LAST_RESULT = None  # BassKernelResults of the most recent run

_NC_CACHE = {}

_MYBIR_DT = {"f32": F32, "f32r": F32R, "bf16": BF16, "f16": F16}
_NP_DT = {"f32": np.float32, "f32r": np.float32, "bf16": ml_dtypes.bfloat16,
          "f16": np.float16}


def _build_nc(w_kind, cache_kind):
    """Build + compile the single-core Bass program (SPMD across 8 cores)."""
    nc = bacc.Bacc("TRN2", target_bir_lowering=False, debug=False,
                   num_devices=N_CORES, enable_asserts=False)

    dt = F32
    dt_w = _MYBIR_DT[w_kind]       # x / weight matmul operands
    dt_c = _MYBIR_DT[cache_kind]   # attention matmul operands

    xT_d = nc.dram_tensor("xT", [NC_I, 128, T], dt_w, kind="ExternalInput").ap()
    wqT_d = nc.dram_tensor("wqT", [NC_I, 128, O], dt_w, kind="ExternalInput").ap()
    wkT_d = nc.dram_tensor("wkT", [NC_I, 128, O], dt_w, kind="ExternalInput").ap()
    wvT_d = nc.dram_tensor("wvT", [NC_I, 128, O], dt_w, kind="ExternalInput").ap()
    woT_d = nc.dram_tensor("woT", [NH, 128, D], dt_w, kind="ExternalInput").ap()
    bq_d = nc.dram_tensor("bq", [O], dt, kind="ExternalInput").ap()
    bk_d = nc.dram_tensor("bk", [O], dt, kind="ExternalInput").ap()
    bv_d = nc.dram_tensor("bv", [O], dt, kind="ExternalInput").ap()
    kT_d = nc.dram_tensor("kT4", [NH, 128, CACHE_POS], dt_c,
                          kind="ExternalInput").ap()
    v_d = nc.dram_tensor("v4", [NH, NC_S, 128, HD + 1], dt_c,
                         kind="ExternalInput").ap()
    y_d = nc.dram_tensor("y", [T, D], dt, kind="ExternalOutput").ap()

    with tile.TileContext(nc) as tc:
        with (
            tc.tile_pool(name="const", bufs=1) as const_pool,
            tc.tile_pool(name="wstream", bufs=3) as w_pool,
            tc.tile_pool(name="kvstream", bufs=4) as kv_pool,
            tc.tile_pool(name="wopool", bufs=4) as wo_pool,
            tc.tile_pool(name="pTpool", bufs=2) as pT_pool,
            tc.tile_pool(name="small", bufs=2) as small_pool,
        ):
            # ---- constants / persistent tiles ----
            ident = const_pool.tile([128, 128], dt)
            make_identity(nc, ident[:])

            xT_sb = const_pool.tile([128, NC_I, T], dt_w)
            nc.sync.dma_start(out=xT_sb[:], in_=xT_d.rearrange("c p t -> p c t"))

            def _bcast(ap_1d):
                return bass.AP(tensor=ap_1d.tensor, offset=ap_1d.offset,
                               ap=[[0, 128]] + [list(p) for p in ap_1d.ap])

            bq_sb = const_pool.tile([128, O], dt)
            bk_sb = const_pool.tile([128, O], dt)
            bv_sb = const_pool.tile([128, O], dt)
            nc.gpsimd.dma_start(out=bq_sb[:], in_=_bcast(bq_d))
            nc.gpsimd.dma_start(out=bk_sb[:], in_=_bcast(bk_d))
            nc.gpsimd.dma_start(out=bv_sb[:], in_=_bcast(bv_d))

            qT_sb = const_pool.tile([128, NH, T], dt_c)      # per head [hd, t]
            kT_new = const_pool.tile([128, NH, T], dt_c)     # per head [hd, t_new]
            v_new = const_pool.tile([128, NH, HD + 1], dt_c)  # [t_new, hd|1]
            aoT_sb = const_pool.tile([128, NH, T], dt_w)     # per head [hd, t]
            y_sb = const_pool.tile([128, D], dt)

            for h in range(NH):
                nc.vector.memset(v_new[:, h, HD:HD + 1], 1.0)

            expb = const_pool.tile([128, 1], F32)
            nc.vector.memset(expb[:], EXP_BIAS)

            with (
                tc.tile_pool(name="tr_psum", bufs=2, space="PSUM") as tr_psum,
                tc.tile_pool(name="proj_psum", bufs=2, space="PSUM") as proj_psum,
                tc.tile_pool(name="kq_psum", bufs=2, space="PSUM") as kq_psum,
                tc.tile_pool(name="av_psum", bufs=2, space="PSUM") as av_psum,
            ):
                # ---- phase A: q/k/v projections ----
                if True:
                    for name, wT_dram, bias_sb in (
                        ("q", wqT_d, bq_sb), ("k", wkT_d, bk_sb),
                        ("v", wvT_d, bv_sb),
                    ):
                        ps = proj_psum.tile([128, O], F32, tag="proj")
                        for g in range(NC_I // 4):
                            wch = w_pool.tile([128, 4, O], dt_w, tag="w")
                            nc.sync.dma_start(
                                out=wch[:],
                                in_=wT_dram[ts(g, 4), :, :].rearrange(
                                    "c p o -> p c o"),
                            )
                            for cc in range(4):
                                c = g * 4 + cc
                                nc.tensor.matmul(
                                    ps[:],
                                    xT_sb[:, c, :],
                                    wch[:, cc, :],
                                    start=(c == 0), stop=(c == NC_I - 1),
                                )
                        # bias add, evict PSUM -> SBUF
                        proj_sb = small_pool.tile([128, O], dt, tag="proj_sb")
                        nc.vector.tensor_add(proj_sb[:], ps[:], bias_sb[:])

                        if name == "v":
                            for h in range(NH):
                                nc.vector.tensor_copy(
                                    v_new[:, h, 0:HD], proj_sb[:, ts(h, HD)])
                        else:
                            dest = qT_sb if name == "q" else kT_new
                            for h in range(NH):
                                tp = tr_psum.tile([128, 128], F32, tag="tr")
                                nc.tensor.transpose(
                                    tp[:], proj_sb[:, ts(h, HD)], ident[:])
                                nc.vector.tensor_copy(dest[:, h, :], tp[:])

                # ---- phase B: attention per head ----
                if True:
                    for h in range(NH):
                        kT_s = kv_pool.tile([128, CACHE_POS], dt_c, tag="kt")
                        nc.sync.dma_start(out=kT_s[:], in_=kT_d[h])
                        v_s = kv_pool.tile([128, NC_S, HD + 1], dt_c, tag="v")
                        nc.sync.dma_start(
                            out=v_s[:], in_=v_d[h].rearrange("c p o -> p c o"))

                        pT = pT_pool.tile([128, S], dt_c, tag="pT")

                        # scores^T in s-chunks of 128, grouped 4 per PSUM bank,
                        # exp()'d on eviction (scale folds in 1/sqrt(hd))
                        for g in range(NC_S // 4):
                            ps = kq_psum.tile([128, 512], F32, tag="kq")
                            for cc in range(4):
                                c = g * 4 + cc
                                nc.tensor.matmul(
                                    ps[:, ts(cc, 128)],
                                    kT_s[:, ts(c, 128)],
                                    qT_sb[:, h, :],
                                    start=True, stop=True,
                                )
                            nc.scalar.activation(
                                pT[:, ts(g, 512)], ps[:],
                                mybir.ActivationFunctionType.Exp,
                                bias=expb[:], scale=SCALE)
                        # 33rd chunk: the freshly appended k rows
                        ps = kq_psum.tile([128, 512], F32, tag="kq")
                        nc.tensor.matmul(
                            ps[:, 0:128], kT_new[:, h, :], qT_sb[:, h, :],
                            start=True, stop=True)
                        nc.scalar.activation(
                            pT[:, CACHE_POS:S], ps[:, 0:128],
                            mybir.ActivationFunctionType.Exp,
                            bias=expb[:], scale=SCALE)

                        # attn @ [v | 1]: accumulate over all 33 s-chunks
                        av = av_psum.tile([128, HD + 1], F32, tag="av")
                        for c in range(NC_S):
                            nc.tensor.matmul(
                                av[:], pT[:, ts(c, 128)], v_s[:, c, :],
                                start=(c == 0), stop=False)
                        nc.tensor.matmul(
                            av[:], pT[:, CACHE_POS:S], v_new[:, h, :],
                            start=False, stop=True)

                        # normalize by the ones-column sum, transpose for wo
                        recip = small_pool.tile([128, 1], F32, tag="recip")
                        nc.vector.reciprocal(recip[:], av[:, HD:HD + 1])
                        ao_n = small_pool.tile([128, HD], dt, tag="ao_n")
                        nc.vector.tensor_scalar_mul(
                            ao_n[:], av[:, 0:HD], recip[:])
                        tp = tr_psum.tile([128, 128], F32, tag="tr")
                        nc.tensor.transpose(tp[:], ao_n[:], ident[:])
                        nc.vector.tensor_copy(aoT_sb[:, h, :], tp[:])

            # ---- phase C: output projection (partial over this core's dims) --
            with tc.tile_pool(name="wo_psum", bufs=1, space="PSUM") as wo_psum:
                ys = []
                for j in range(8):
                    yj = wo_psum.tile([128, 512], F32, tag=f"y{j}",
                                      name=f"yps{j}")
                    ys.append(yj)
                for c in range(NH):
                    woch = wo_pool.tile([128, D], dt_w, tag="wo")
                    nc.sync.dma_start(out=woch[:], in_=woT_d[c])
                    for j in range(8):
                        nc.tensor.matmul(
                            ys[j][:],
                            aoT_sb[:, c, :],
                            woch[:, ts(j, 512)],
                            start=(c == 0), stop=(c == NH - 1),
                        )
                for j in range(8):
                    nc.vector.tensor_copy(y_sb[:, ts(j, 512)], ys[j][:])

            nc.sync.dma_start(out=y_d[:], in_=y_sb[:])

    nc.compile()
    return nc


def _prep_core_inputs(c, x, wq_w, wq_b, wk_w, wk_b, wv_w, wv_b, wo_w,
                      k_cache, v_cache, w_np, c_np):
    isl = slice(c * O, (c + 1) * O)
    hsl = slice(c * NH, (c + 1) * NH)
    f32 = np.float32

    xT = np.ascontiguousarray(x[0].T, dtype=w_np).reshape(NC_I, 128, T)
    wqT = np.ascontiguousarray(wq_w[isl, :].T, dtype=w_np).reshape(NC_I, 128, O)
    wkT = np.ascontiguousarray(wk_w[isl, :].T, dtype=w_np).reshape(NC_I, 128, O)
    wvT = np.ascontiguousarray(wv_w[isl, :].T, dtype=w_np).reshape(NC_I, 128, O)
    woT = np.ascontiguousarray(wo_w[:, isl].T, dtype=w_np).reshape(NH, 128, D)

    # k-cache per head, transposed to [hd, s] (only the pre-existing rows)
    kT4 = np.ascontiguousarray(
        k_cache[:CACHE_POS, hsl, :].transpose(1, 2, 0), dtype=c_np)
    # v-cache per head as [s_chunk, s_local, hd] with a ones column appended
    v4 = np.empty((NH, NC_S, 128, HD + 1), dtype=c_np)
    v4[:, :, :, 0:HD] = v_cache[:CACHE_POS, hsl, :].reshape(
        NC_S, 128, NH, HD).transpose(2, 0, 1, 3)
    v4[:, :, :, HD] = 1.0

    return {
        "xT": xT, "wqT": wqT, "wkT": wkT, "wvT": wvT, "woT": woT,
        "bq": np.ascontiguousarray(wq_b[isl], dtype=f32),
        "bk": np.ascontiguousarray(wk_b[isl], dtype=f32),
        "bv": np.ascontiguousarray(wv_b[isl], dtype=f32),
        "kT4": kT4, "v4": v4,
    }


def kernel(x, wq_w, wq_b, wk_w, wk_b, wv_w, wv_b, wo_w, wo_b,
           k_cache, v_cache, pos, cache_pos, **_ignored):
    global LAST_RESULT
    assert int(cache_pos) == CACHE_POS, "kernel hardcodes cache_pos=4096"

    key = (W_DT, CACHE_DT)
    if key not in _NC_CACHE:
        _NC_CACHE[key] = _build_nc(W_DT, CACHE_DT)
    nc = _NC_CACHE[key]

    x = np.asarray(x, dtype=np.float32)
    w_np, c_np = _NP_DT[W_DT], _NP_DT[CACHE_DT]
    in_maps = [
        _prep_core_inputs(c, x, np.asarray(wq_w), np.asarray(wq_b),
                          np.asarray(wk_w), np.asarray(wk_b),
                          np.asarray(wv_w), np.asarray(wv_b),
                          np.asarray(wo_w), np.asarray(k_cache),
                          np.asarray(v_cache), w_np, c_np)
        for c in range(N_CORES)
    ]

    kwargs = {}
    if TRACE:
        _install_profile_hook()
        kwargs = {"trace": True}
    res = run_bass_kernel_spmd(nc, in_maps, list(range(N_CORES)), **kwargs)
    LAST_RESULT = res

    y = res.results[0]["y"].astype(np.float64)
    for c in range(1, N_CORES):
        y = y + res.results[c]["y"].astype(np.float64)
    y = (y + np.asarray(wo_b, dtype=np.float64)).astype(np.float32)
    return y.reshape(B, T, D)


def _install_profile_hook():
    """Register the axon NTFF profiling hook (the agent image lacks
    antenv.axon_hooks; mirror what trn_agent_boot.trn_boot would do)."""
    import contextlib
    import ctypes
    import types

    import antenv

    if "antenv.axon_hooks" in sys.modules:
        return
    mod = types.ModuleType("antenv.axon_hooks")
    holder = {}
    mod.set_axon_ntff_profile_hook = lambda h: holder.__setitem__("h", h)
    mod.get_axon_ntff_profile_hook = lambda: holder.get("h")
    sys.modules["antenv.axon_hooks"] = mod
    antenv.axon_hooks = mod

    lib = ctypes.CDLL("/opt/axon/libaxon_pjrt.so")
    if not hasattr(lib, "axon_start_nrt_profile"):
        return
    lib.axon_start_nrt_profile.argtypes = [
        ctypes.POINTER(ctypes.c_int64), ctypes.c_size_t]
    lib.axon_start_nrt_profile.restype = ctypes.c_int64
    lib.axon_stop_nrt_profile.argtypes = [ctypes.c_char_p]
    lib.axon_stop_nrt_profile.restype = ctypes.c_int64

    @contextlib.contextmanager
    def _hook(output_dir, device_ids):
        import jax
        jax.devices()
        if device_ids:
            ids = (ctypes.c_int64 * len(device_ids))(*device_ids)
            rc = lib.axon_start_nrt_profile(ids, len(device_ids))
        else:
            rc = lib.axon_start_nrt_profile(None, 0)
        if rc != 0:
            raise RuntimeError(f"axon_start_nrt_profile rc={rc}")
        try:
            yield
        finally:
            n = lib.axon_stop_nrt_profile(str(output_dir).encode())
            if n <= 0:
                print(f"profile: rc={n} (no ntff written) in {output_dir}")

    mod.set_axon_ntff_profile_hook(_hook)


# revision 21
# speedup vs baseline: 1.1732x; 1.1732x over previous
"""Trainium2 Bass kernel for nn_AttentionWithCache (decode attention with KV cache).

Full-input contract: kernel(**inputs) takes the unsharded numpy inputs and
returns the full [1, 128, 4096] output. Internally shards tensor-parallel
over heads across 8 NeuronCores (4 heads each), runs a Bass/Tile kernel via
run_bass_kernel_spmd, and reduces the wo partial sums on gather.

Key algebraic simplification: the reference applies RoPE at a single scalar
position `pos` to BOTH q and the whole live k-cache. A per-(i, i+64) plane
rotation by the same angle on both operands of a dot product leaves the dot
product unchanged, and v is never rotated — so attention scores (and hence
the output) are mathematically identical without RoPE. The kernel skips it.

Softmax skips max-subtraction: scores are ~N(0, 1.65^2) (|s| < ~10), so
exp() cannot overflow in fp32 and softmax is shift-invariant. The softmax
denominator comes for free from a ones-column appended to v (the attn@v
matmul's extra output column is the row sum of the probabilities).
"""

import sys

if "/opt/trn_rl_repo" not in sys.path:
    sys.path.insert(0, "/opt/trn_rl_repo")

import ml_dtypes
import numpy as np

import concourse.bass as bass
import concourse.mybir as mybir
import concourse.tile as tile
from concourse import bacc
from concourse.bass import ts
from concourse.bass_utils import run_bass_kernel_spmd
from concourse.masks import make_identity

# Problem shapes (hardcoded per contract).
B, T, D = 1, 128, 4096
H, HD = 32, 128
CACHE_POS = 4096
S = CACHE_POS + T            # 4224 live cache rows
N_CORES = 8
NH = H // N_CORES            # 4 heads per core
O = NH * HD                  # 512 projection out-dims per core
NC_I = D // 128              # 32 contraction chunks for projections
NC_S = CACHE_POS // 128      # 32 old-cache s-chunks (the 33rd chunk is new k/v)
SCALE = 1.0 / float(np.sqrt(HD))
# Constant subtracted inside exp() (cancels exactly in the softmax ratio).
# Raw scores reach ~±18.5; fp16 exp overflows at 11.09, so shift down.
EXP_BIAS = -13.0

F32 = mybir.dt.float32
F32R = mybir.dt.float32r
BF16 = mybir.dt.bfloat16
F16 = mybir.dt.float16

# Precision config.
#   W_DT:     dtype of x + projection/wo weights as matmul operands
#             ('f32r' = tf32-like, 4B DMA; 'bf16' = 2B DMA; 'f32' exact 2-pass)
#   CACHE_DT: dtype of k/v caches + q/probs as attention matmul operands
W_DT = "f32r"
CACHE_DT = "f32"

TRACE = False       # set by test.py for profiling runs
LAST_RESULT = None  # BassKernelResults of the most recent run

_NC_CACHE = {}

_MYBIR_DT = {"f32": F32, "f32r": F32R, "bf16": BF16, "f16": F16}
_NP_DT = {"f32": np.float32, "f32r": np.float32, "bf16": ml_dtypes.bfloat16,
          "f16": np.float16}


def _build_nc(w_kind, cache_kind):
    """Build + compile the single-core Bass program (SPMD across 8 cores)."""
    nc = bacc.Bacc("TRN2", target_bir_lowering=False, debug=False,
                   num_devices=N_CORES, enable_asserts=False)

    dt = F32
    dt_w = _MYBIR_DT[w_kind]       # x / weight matmul operands
    dt_c = _MYBIR_DT[cache_kind]   # attention matmul operands

    xT_d = nc.dram_tensor("xT", [128, NC_I, T], dt_w, kind="ExternalInput").ap()
    wqT_d = nc.dram_tensor("wqT", [NC_I, 128, O], dt_w, kind="ExternalInput").ap()
    wkT_d = nc.dram_tensor("wkT", [NC_I, 128, O], dt_w, kind="ExternalInput").ap()
    wvT_d = nc.dram_tensor("wvT", [NC_I, 128, O], dt_w, kind="ExternalInput").ap()
    woT_d = nc.dram_tensor("woT", [NH, 128, D], dt_w, kind="ExternalInput").ap()
    bq_d = nc.dram_tensor("bq", [O], dt, kind="ExternalInput").ap()
    bk_d = nc.dram_tensor("bk", [O], dt, kind="ExternalInput").ap()
    bv_d = nc.dram_tensor("bv", [O], dt, kind="ExternalInput").ap()
    kT_d = nc.dram_tensor("kT4", [NH, 128, CACHE_POS], dt_c,
                          kind="ExternalInput").ap()
    v_d = nc.dram_tensor("v4", [NH, 128, NC_S, HD + 1], dt_c,
                         kind="ExternalInput").ap()
    y_d = nc.dram_tensor("y", [T, D], dt, kind="ExternalOutput").ap()

    with tile.TileContext(nc) as tc:
        with (
            tc.tile_pool(name="const", bufs=1) as const_pool,
            tc.tile_pool(name="wstream", bufs=3) as w_pool,
            tc.tile_pool(name="kvstream", bufs=4) as kv_pool,
            tc.tile_pool(name="wopool", bufs=4) as wo_pool,
            tc.tile_pool(name="pTpool", bufs=2) as pT_pool,
            tc.tile_pool(name="small", bufs=2) as small_pool,
        ):
            # ---- constants / persistent tiles ----
            ident = const_pool.tile([128, 128], dt)
            make_identity(nc, ident[:])

            xT_sb = const_pool.tile([128, NC_I, T], dt_w)
            nc.sync.dma_start(out=xT_sb[:], in_=xT_d)

            def _bcast(ap_1d):
                return bass.AP(tensor=ap_1d.tensor, offset=ap_1d.offset,
                               ap=[[0, 128]] + [list(p) for p in ap_1d.ap])

            bq_sb = const_pool.tile([128, O], dt)
            bk_sb = const_pool.tile([128, O], dt)
            bv_sb = const_pool.tile([128, O], dt)
            nc.gpsimd.dma_start(out=bq_sb[:], in_=_bcast(bq_d))
            nc.gpsimd.dma_start(out=bk_sb[:], in_=_bcast(bk_d))
            nc.gpsimd.dma_start(out=bv_sb[:], in_=_bcast(bv_d))

            qT_sb = const_pool.tile([128, NH, T], dt_c)      # per head [hd, t]
            kT_new = const_pool.tile([128, NH, T], dt_c)     # per head [hd, t_new]
            v_new = const_pool.tile([128, NH, HD + 1], dt_c)  # [t_new, hd|1]
            aoT_sb = const_pool.tile([128, NH, T], dt_w)     # per head [hd, t]
            y_sb = const_pool.tile([128, D], dt)

            for h in range(NH):
                nc.vector.memset(v_new[:, h, HD:HD + 1], 1.0)

            expb = const_pool.tile([128, 1], F32)
            nc.vector.memset(expb[:], EXP_BIAS)

            with tc.tile_pool(name="tr_psum", bufs=2, space="PSUM") as tr_psum:
                # ---- phase A: q/k/v projections ----
                with tc.tile_pool(name="proj_psum", bufs=2,
                                  space="PSUM") as proj_psum:
                    for name, wT_dram, bias_sb in (
                        ("q", wqT_d, bq_sb), ("k", wkT_d, bk_sb),
                        ("v", wvT_d, bv_sb),
                    ):
                        ps = proj_psum.tile([128, O], F32, tag="proj")
                        for g in range(NC_I // 4):
                            wch = w_pool.tile([128, 4, O], dt_w, tag="w")
                            nc.sync.dma_start(
                                out=wch[:],
                                in_=wT_dram[ts(g, 4), :, :].rearrange(
                                    "c p o -> p c o"),
                            )
                            for cc in range(4):
                                c = g * 4 + cc
                                nc.tensor.matmul(
                                    ps[:],
                                    xT_sb[:, c, :],
                                    wch[:, cc, :],
                                    start=(c == 0), stop=(c == NC_I - 1),
                                )
                        # bias add, evict PSUM -> SBUF
                        proj_sb = small_pool.tile([128, O], dt, tag="proj_sb")
                        nc.vector.tensor_add(proj_sb[:], ps[:], bias_sb[:])

                        if name == "v":
                            for h in range(NH):
                                nc.vector.tensor_copy(
                                    v_new[:, h, 0:HD], proj_sb[:, ts(h, HD)])
                        else:
                            dest = qT_sb if name == "q" else kT_new
                            for h in range(NH):
                                tp = tr_psum.tile([128, 128], F32, tag="tr")
                                nc.tensor.transpose(
                                    tp[:], proj_sb[:, ts(h, HD)], ident[:])
                                nc.vector.tensor_copy(dest[:, h, :], tp[:])

                # ---- phase B: attention per head ----
                with (
                    tc.tile_pool(name="kq_psum", bufs=3, space="PSUM") as kq_psum,
                    tc.tile_pool(name="av_psum", bufs=2, space="PSUM") as av_psum,
                ):
                    for h in range(NH):
                        kT_s = kv_pool.tile([128, CACHE_POS], dt_c, tag="kt")
                        nc.sync.dma_start(out=kT_s[:], in_=kT_d[h])
                        v_s = kv_pool.tile([128, NC_S, HD + 1], dt_c, tag="v")
                        nc.sync.dma_start(out=v_s[:], in_=v_d[h])

                        pT = pT_pool.tile([128, S], dt_c, tag="pT")

                        # scores^T in s-chunks of 128, grouped 4 per PSUM bank,
                        # exp()'d on eviction (scale folds in 1/sqrt(hd))
                        for g in range(NC_S // 4):
                            ps = kq_psum.tile([128, 512], F32, tag="kq")
                            for cc in range(4):
                                c = g * 4 + cc
                                nc.tensor.matmul(
                                    ps[:, ts(cc, 128)],
                                    kT_s[:, ts(c, 128)],
                                    qT_sb[:, h, :],
                                    start=True, stop=True,
                                )
                            nc.scalar.activation(
                                pT[:, ts(g, 512)], ps[:],
                                mybir.ActivationFunctionType.Exp,
                                bias=expb[:], scale=SCALE)
                        # 33rd chunk: the freshly appended k rows
                        ps = kq_psum.tile([128, 512], F32, tag="kq")
                        nc.tensor.matmul(
                            ps[:, 0:128], kT_new[:, h, :], qT_sb[:, h, :],
                            start=True, stop=True)
                        nc.scalar.activation(
                            pT[:, CACHE_POS:S], ps[:, 0:128],
                            mybir.ActivationFunctionType.Exp,
                            bias=expb[:], scale=SCALE)

                        # attn @ [v | 1]: accumulate over all 33 s-chunks
                        av = av_psum.tile([128, HD + 1], F32, tag="av")
                        for c in range(NC_S):
                            nc.tensor.matmul(
                                av[:], pT[:, ts(c, 128)], v_s[:, c, :],
                                start=(c == 0), stop=False)
                        nc.tensor.matmul(
                            av[:], pT[:, CACHE_POS:S], v_new[:, h, :],
                            start=False, stop=True)

                        # normalize by the ones-column sum, transpose for wo
                        recip = small_pool.tile([128, 1], F32, tag="recip")
                        nc.vector.reciprocal(recip[:], av[:, HD:HD + 1])
                        ao_n = small_pool.tile([128, HD], dt, tag="ao_n")
                        nc.vector.tensor_scalar_mul(
                            ao_n[:], av[:, 0:HD], recip[:])
                        tp = tr_psum.tile([128, 128], F32, tag="tr")
                        nc.tensor.transpose(tp[:], ao_n[:], ident[:])
                        nc.vector.tensor_copy(aoT_sb[:, h, :], tp[:])

            # ---- phase C: output projection (partial over this core's dims) --
            with tc.tile_pool(name="wo_psum", bufs=1, space="PSUM") as wo_psum:
                ys = []
                for j in range(8):
                    yj = wo_psum.tile([128, 512], F32, tag=f"y{j}",
                                      name=f"yps{j}")
                    ys.append(yj)
                for c in range(NH):
                    woch = wo_pool.tile([128, D], dt_w, tag="wo")
                    nc.sync.dma_start(out=woch[:], in_=woT_d[c])
                    for j in range(8):
                        nc.tensor.matmul(
                            ys[j][:],
                            aoT_sb[:, c, :],
                            woch[:, ts(j, 512)],
                            start=(c == 0), stop=(c == NH - 1),
                        )
                for j in range(8):
                    nc.vector.tensor_copy(y_sb[:, ts(j, 512)], ys[j][:])

            nc.sync.dma_start(out=y_d[:], in_=y_sb[:])

    nc.compile()
    return nc


def _prep_core_inputs(c, x, wq_w, wq_b, wk_w, wk_b, wv_w, wv_b, wo_w,
                      k_cache, v_cache, w_np, c_np):
    isl = slice(c * O, (c + 1) * O)
    hsl = slice(c * NH, (c + 1) * NH)
    f32 = np.float32

    xT = np.ascontiguousarray(
        x[0].T.reshape(NC_I, 128, T).transpose(1, 0, 2), dtype=w_np)
    wqT = np.ascontiguousarray(wq_w[isl, :].T, dtype=w_np).reshape(NC_I, 128, O)
    wkT = np.ascontiguousarray(wk_w[isl, :].T, dtype=w_np).reshape(NC_I, 128, O)
    wvT = np.ascontiguousarray(wv_w[isl, :].T, dtype=w_np).reshape(NC_I, 128, O)
    woT = np.ascontiguousarray(wo_w[:, isl].T, dtype=w_np).reshape(NH, 128, D)

    # k-cache per head, transposed to [hd, s] (only the pre-existing rows)
    kT4 = np.ascontiguousarray(
        k_cache[:CACHE_POS, hsl, :].transpose(1, 2, 0), dtype=c_np)
    # v-cache per head as [s_chunk, s_local, hd] with a ones column appended
    v4 = np.empty((NH, 128, NC_S, HD + 1), dtype=c_np)
    v4[:, :, :, 0:HD] = v_cache[:CACHE_POS, hsl, :].reshape(
        NC_S, 128, NH, HD).transpose(2, 1, 0, 3)
    v4[:, :, :, HD] = 1.0

    return {
        "xT": xT, "wqT": wqT, "wkT": wkT, "wvT": wvT, "woT": woT,
        "bq": np.ascontiguousarray(wq_b[isl], dtype=f32),
        "bk": np.ascontiguousarray(wk_b[isl], dtype=f32),
        "bv": np.ascontiguousarray(wv_b[isl], dtype=f32),
        "kT4": kT4, "v4": v4,
    }


def kernel(x, wq_w, wq_b, wk_w, wk_b, wv_w, wv_b, wo_w, wo_b,
           k_cache, v_cache, pos, cache_pos, **_ignored):
    global LAST_RESULT
    assert int(cache_pos) == CACHE_POS, "kernel hardcodes cache_pos=4096"

    key = (W_DT, CACHE_DT)
    if key not in _NC_CACHE:
        _NC_CACHE[key] = _build_nc(W_DT, CACHE_DT)
    nc = _NC_CACHE[key]

    x = np.asarray(x, dtype=np.float32)
    w_np, c_np = _NP_DT[W_DT], _NP_DT[CACHE_DT]
    in_maps = [
        _prep_core_inputs(c, x, np.asarray(wq_w), np.asarray(wq_b),
                          np.asarray(wk_w), np.asarray(wk_b),
                          np.asarray(wv_w), np.asarray(wv_b),
                          np.asarray(wo_w), np.asarray(k_cache),
                          np.asarray(v_cache), w_np, c_np)
        for c in range(N_CORES)
    ]

    kwargs = {}
    if TRACE:
        _install_profile_hook()
        kwargs = {"trace": True}
    res = run_bass_kernel_spmd(nc, in_maps, list(range(N_CORES)), **kwargs)
    LAST_RESULT = res

    y = res.results[0]["y"].astype(np.float64)
    for c in range(1, N_CORES):
        y = y + res.results[c]["y"].astype(np.float64)
    y = (y + np.asarray(wo_b, dtype=np.float64)).astype(np.float32)
    return y.reshape(B, T, D)


def _install_profile_hook():
    """Register the axon NTFF profiling hook (the agent image lacks
    antenv.axon_hooks; mirror what trn_agent_boot.trn_boot would do)."""
    import contextlib
    import ctypes
    import types

    import antenv

    if "antenv.axon_hooks" in sys.modules:
        return
    mod = types.ModuleType("antenv.axon_hooks")
    holder = {}
    mod.set_axon_ntff_profile_hook = lambda h: holder.__setitem__("h", h)
    mod.get_axon_ntff_profile_hook = lambda: holder.get("h")
    sys.modules["antenv.axon_hooks"] = mod
    antenv.axon_hooks = mod

    lib = ctypes.CDLL("/opt/axon/libaxon_pjrt.so")
    if not hasattr(lib, "axon_start_nrt_profile"):
        return
    lib.axon_start_nrt_profile.argtypes = [
        ctypes.POINTER(ctypes.c_int64), ctypes.c_size_t]
    lib.axon_start_nrt_profile.restype = ctypes.c_int64
    lib.axon_stop_nrt_profile.argtypes = [ctypes.c_char_p]
    lib.axon_stop_nrt_profile.restype = ctypes.c_int64

    @contextlib.contextmanager
    def _hook(output_dir, device_ids):
        import jax
        jax.devices()
        if device_ids:
            ids = (ctypes.c_int64 * len(device_ids))(*device_ids)
            rc = lib.axon_start_nrt_profile(ids, len(device_ids))
        else:
            rc = lib.axon_start_nrt_profile(None, 0)
        if rc != 0:
            raise RuntimeError(f"axon_start_nrt_profile rc={rc}")
        try:
            yield
        finally:
            n = lib.axon_stop_nrt_profile(str(output_dir).encode())
            if n <= 0:
                print(f"profile: rc={n} (no ntff written) in {output_dir}")

    mod.set_axon_ntff_profile_hook(_hook)
